# revision 28
# baseline (speedup 1.0000x reference)
"""GPT2 eager causal attention (B=2, S=2048, D=1024, H=16, HD=64) on 8 TRN2 NeuronCores.

Sharding (data + head/tensor parallel): core c -> (batch b = c//4, head-group
g = c%4) -- 4 heads per group; each quad (same batch) exchanges attention
outputs with a small bf16 AllToAll and computes the full c_proj locally for
its own token slices (no ReduceScatter, no fp32 partials).

Per-core pipeline:
  0. host pre-transposes x -> xT[d, s] and pre-interleaves every weight into
     its SBUF layout, so all device DMAs are large and contiguous.
  1. per 512-token chunk ch: QT/KT row-tiles and V strips via PE matmuls
     (d-contraction over 8 k-tiles), interleaved with...
  2. attention for chunk ch, head-pair t: score tiles for both heads of the
     pair go into one [128,1024] 2-bank PSUM tile via two row-group-packed
     K=64 matmuls (rows 0:64 / 64:128 -> concurrent on the PE sub-arrays);
     ONE exp covers both heads; diagonal k-tiles exp only the causal suffix
     and the st/ot matmuls skip the masked prefix columns entirely.
     V carries a ones-column per head so ot row 64 is the softmax denominator;
     reciprocal straight from PSUM, broadcast to 128 partitions with a K=2
     sel-matmul, normalize on DVE.
  3. OT chunk [256, 512]bf16 -> DRAM -> AllToAll over the quad (each core
     receives the full-model OT columns for its 128-token slice) -> local
     c_proj with the full w_proj -> fp32 out rows.
"""
from contextlib import ExitStack

import ml_dtypes
import numpy as np

import concourse.bacc as bacc
import concourse.mybir as mybir
import concourse.tile as tile
from concourse.bass import AP
from concourse.bass_utils import run_bass_kernel_spmd

F32 = mybir.dt.float32
BF16 = mybir.dt.bfloat16

B, S, D, H, HD = 2, 2048, 1024, 16, 64
N_CORES = 8
HG = 4               # heads per group (per core)
DG = HG * HD         # 256 q/k channels per group
VW = HG * (HD + 1)   # 260: 64 v-cols + 1 ones-col per head
NK = D // 128        # 8 contraction tiles over d
CH = 512             # q-chunk (one PSUM bank of fp32)
NCH = S // CH        # 4
NRT = DG // 128      # 2 channel row-tiles per group
WARMUP_MM = 40       # dummy matmuls to lift the PE HAM throttle before real work


def _build(has_bv: bool, has_bp: bool, has_bqk: bool = False, dump_ot: bool = False):
    nc = bacc.Bacc("TRN2", target_bir_lowering=False, debug=False, num_devices=N_CORES)
    dbg_d = nc.dram_tensor("dbg", [128, NRT * S], F32, kind="ExternalOutput").ap() if dump_ot else None

    xt_d = nc.dram_tensor("xt", [128, NCH * NK * CH], BF16, kind="ExternalInput").ap()
    wq_d = nc.dram_tensor("wq", [128, NK * DG], BF16, kind="ExternalInput").ap()
    wk_d = nc.dram_tensor("wk", [128, NK * DG], BF16, kind="ExternalInput").ap()
    wv_d = nc.dram_tensor("wv", [128, NK * VW], BF16, kind="ExternalInput").ap()
    wp_d = nc.dram_tensor("wp", [128, NK * D], BF16, kind="ExternalInput").ap()
    bq_d = nc.dram_tensor("bq", [128, NRT], F32, kind="ExternalInput").ap()
    bk_d = nc.dram_tensor("bk", [128, NRT], F32, kind="ExternalInput").ap()
    bv_d = nc.dram_tensor("bv", [128, NRT], F32, kind="ExternalInput").ap()
    bp_d = nc.dram_tensor("bp", [128, D], F32, kind="ExternalInput").ap()
    mk_d = nc.dram_tensor("masks", [128, 512], BF16, kind="ExternalInput").ap()
    out_d = nc.dram_tensor("out", [CH, D], F32, kind="ExternalOutput").ap()

    EXP = mybir.ActivationFunctionType.Exp
    IDENT = mybir.ActivationFunctionType.Identity

    with ExitStack() as ctx:
        tc = ctx.enter_context(tile.TileContext(nc))
        wpool = ctx.enter_context(tc.tile_pool(name="w", bufs=1))
        big = ctx.enter_context(tc.tile_pool(name="big", bufs=1))
        qkvp = ctx.enter_context(tc.tile_pool(name="qkv", bufs=1))
        stp = ctx.enter_context(tc.tile_pool(name="stx", bufs=4))
        nrm = ctx.enter_context(tc.tile_pool(name="nrm", bufs=2))
        cpj = ctx.enter_context(tc.tile_pool(name="cpj", bufs=2))
        outp = ctx.enter_context(tc.tile_pool(name="outp", bufs=3))
        ps_mm = ctx.enter_context(tc.tile_pool(name="psmm", bufs=2, space="PSUM"))
        ps_st = ctx.enter_context(tc.tile_pool(name="psst", bufs=2, space="PSUM"))
        ps_ot = ctx.enter_context(tc.tile_pool(name="psot", bufs=1, space="PSUM"))
        dram = ctx.enter_context(tc.tile_pool(name="dram", bufs=1, space="DRAM"))

        # ---- tiny constants first so the PE warm-up can start immediately
        # mk_sb cols 0:128 = triangular causal mask; rows 0:2 cols 128:256 =
        # the sel pattern that broadcasts rden row 0 -> partitions 0:64 and
        # row 1 -> partitions 64:128 via a K=2 matmul.
        mk_sb = wpool.tile([128, 512], BF16)
        nc.sync.dma_start(mk_sb[:], mk_d[:])
        if WARMUP_MM:
            warm_ps = ps_mm.tile([128, 512], F32, tag="ps", name="warm_ps")
            for i in range(WARMUP_MM):
                nc.tensor.matmul(
                    warm_ps[:, 0:256], mk_sb[:, 256:384], mk_sb[:, 0:256],
                    start=True, stop=True,
                )

        # ---- weights / x strips (host already laid out in SBUF order)
        xt_sb = big.tile([128, NCH * NK * CH], BF16, name="xt_sb")
        xt3 = xt_sb.rearrange("p (k c) -> p k c", k=NK)
        wq_sb = wpool.tile([128, NK * DG], BF16)
        wk_sb = wpool.tile([128, NK * DG], BF16)
        wv_sb = wpool.tile([128, NK * VW], BF16)
        wp_sb = wpool.tile([128, NK * D], BF16)
        bq_sb = wpool.tile([128, NRT], F32) if has_bqk else None
        bk_sb = wpool.tile([128, NRT], F32) if has_bqk else None
        bv_sb = wpool.tile([128, NRT], F32) if has_bv else None
        bp_sb = wpool.tile([128, D], F32) if has_bp else None

        def load_x_chunk(ch, eng):
            # 4 sub-DMAs so the chunk spreads across parallel DMA queues
            for q in range(4):
                eng.dma_start(
                    xt3[:, 2 * q:2 * q + 2, ch * CH:(ch + 1) * CH],
                    xt_d[:, ch * NK * CH + 2 * q * CH: ch * NK * CH + (2 * q + 2) * CH],
                )

        # sync HWDGE carries chunk 0 + qkv weights; the scalar HWDGE queue
        # (idle until the first exp) carries the rest -> 2x prologue bandwidth
        load_x_chunk(0, nc.sync)
        nc.sync.dma_start(wq_sb[:], wq_d[:])
        nc.sync.dma_start(wk_sb[:], wk_d[:])
        nc.sync.dma_start(wv_sb[:], wv_d[:])
        for ch in range(1, NCH):
            load_x_chunk(ch, nc.scalar)
        nc.scalar.dma_start(wp_sb[:, :NK * D // 2], wp_d[:, :NK * D // 2])
        nc.scalar.dma_start(wp_sb[:, NK * D // 2:], wp_d[:, NK * D // 2:])
        if has_bqk:
            nc.sync.dma_start(bq_sb[:], bq_d[:])
            nc.sync.dma_start(bk_sb[:], bk_d[:])
        if has_bv:
            nc.sync.dma_start(bv_sb[:], bv_d[:])
        if has_bp:
            nc.sync.dma_start(bp_sb[:], bp_d[:])

        # ---- persistent SBUF tensors
        QT = [qkvp.tile([128, S], BF16, name=f"qT{rt}") for rt in range(NRT)]
        KT = [qkvp.tile([128, S], BF16, name=f"kT{rt}") for rt in range(NRT)]
        V = [qkvp.tile([128, VW], BF16, tag=f"v{st}", name=f"v{st}") for st in range(S // 128)]
        OT = [big.tile([128, S], BF16, name=f"OT{t}") for t in range(NRT)]

        def qkt_units(ch):
            """QT/KT row-tiles for chunk ch as a generator of small PE bursts."""
            for store, w_sb, b_sb, nm in ((KT, wk_sb, bk_sb, "k"), (QT, wq_sb, bq_sb, "q")):
                for rt in range(NRT):
                    ps = ps_mm.tile([128, CH], F32, tag="ps", name=f"ps{nm}{rt}_{ch}")
                    for kt in range(NK):
                        nc.tensor.matmul(
                            ps[:],
                            w_sb[:, kt * DG + rt * 128: kt * DG + (rt + 1) * 128],
                            xt3[:, kt, ch * CH:(ch + 1) * CH],
                            start=(kt == 0), stop=(kt == NK - 1),
                        )
                        if kt % 2 == 1:
                            yield
                    dst = store[rt][:, ch * CH:(ch + 1) * CH]
                    if has_bqk:
                        nc.scalar.activation(dst, ps[:], IDENT, bias=b_sb[:, rt:rt + 1])
                    else:
                        # scalar engine: it idles during qkv production and
                        # this keeps the DVE free for the normalize chain
                        nc.scalar.copy(dst, ps[:])
                    yield

        def v_units(ch):
            """V strips for chunk ch as a generator of small PE bursts."""
            for st in range(4 * ch, 4 * ch + 4):
                ps = ps_mm.tile([128, CH], F32, tag="ps", name=f"psv{st}")
                for kt in range(NK):
                    nc.tensor.matmul(
                        ps[:, :VW],
                        xt3[:, kt, st * 128:(st + 1) * 128],
                        wv_sb[:, kt * VW:(kt + 1) * VW],
                        start=(kt == 0), stop=(kt == NK - 1),
                    )
                    if kt % 2 == 1:
                        yield
                vt = V[st]
                nc.vector.tensor_copy(vt[:], ps[:, :VW])
                for hl in range(HG):
                    ones_col = vt[:, hl * (HD + 1) + HD: (hl + 1) * (HD + 1)].bitcast(mybir.dt.uint16)
                    nc.vector.memset(ones_col, 0x3F80)  # bf16 1.0
                yield

        def drain(gen):
            for _ in gen:
                pass

        # DRAM staging for the per-chunk AllGather: each core contributes its
        # [256, 512] bf16 OT chunk and receives the full-model [1024, 512];
        # it then loads only the 128-token column slice it owns (dynamic
        # offset from the device id).
        ag_ins, ag_outs = [], []
        for ch in range(NCH):
            ai = dram.tile([DG, CH], BF16, tag=f"agi{ch}", name=f"ag_in{ch}")
            ao = dram.tile([HG * DG, CH], BF16, tag=f"ago{ch}", name=f"ag_out{ch}")
            ag_ins.append(ai)
            ag_outs.append(ao)
        pid = nc.sync.partition_id()
        goff = (pid % HG) * 128  # my token-block column offset in the gathered chunk

        def att_pair(ch, t, filler=None, fill_rate=2):
            """Attention for chunk ch, head pair t (heads 2t, 2t+1).
            After each k-tile step, pulls up to fill_rate units of independent
            PE work from `filler` -- these absorb the PE's exp-wait stalls so
            the HAM throttle never re-engages mid-attention."""
            q0 = ch * CH
            nkt = 4 * (ch + 1)
            ot_ps = {}
            ot_ps[0] = ps_ot.tile([HD + 1, CH], F32, tag="otA", name=f"otA{ch}_{t}")
            ot_ps[1] = ps_ot.tile([HD + 1, CH], F32, tag="otB", name=f"otB{ch}_{t}")
            for kt in range(nkt):
                d = kt - 4 * ch
                d0 = max(d, 0) * 128
                st_ps = ps_st.tile([128, 2 * CH], F32, tag="st", name=f"st{ch}_{t}_{kt}")
                st_sb = stp.tile([128, 2 * CH], BF16, tag="stsb", name=f"se{ch}_{t}_{kt}")
                diag = d >= 0
                for hh in range(2):  # row-group packed pair: concurrent on PE
                    off = 64 * hh
                    nc.tensor.matmul(
                        st_ps[:, hh * CH + d0: (hh + 1) * CH],
                        KT[t][off:off + 64, kt * 128:(kt + 1) * 128],
                        QT[t][off:off + 64, q0 + d0: q0 + CH],
                        start=True, stop=not diag,
                    )
                if diag:
                    # causal mask: PE-accumulate -1e9 onto the diagonal block
                    # (identity @ maskneg), so exp gives exact zeros and the
                    # DVE stays out of the exp->ot chain
                    for hh in range(2):
                        nc.tensor.matmul(
                            st_ps[:, hh * CH + d0: hh * CH + d0 + 128],
                            mk_sb[:, 256:384],
                            mk_sb[:, 0:128],
                            start=False, stop=True,
                        )
                if d <= 0:
                    # one exp covers both heads' 512-column halves
                    nc.scalar.activation(st_sb[:], st_ps[:], EXP, scale=0.125)
                else:
                    for hh in range(2):
                        nc.scalar.activation(
                            st_sb[:, hh * CH + d0: (hh + 1) * CH],
                            st_ps[:, hh * CH + d0: (hh + 1) * CH],
                            EXP, scale=0.125,
                        )
                for hh in range(2):
                    hl = 2 * t + hh
                    nc.tensor.matmul(
                        ot_ps[hh][:, d0:],
                        V[kt][:, hl * (HD + 1):(hl + 1) * (HD + 1)],
                        st_sb[:, hh * CH + d0:(hh + 1) * CH],
                        start=(kt == 0), stop=(kt == nkt - 1),
                    )
                if filler is not None:
                    for _ in range(fill_rate):
                        if next(filler, StopIteration) is StopIteration:
                            filler = None
                            break
            # normalize: row 64 of each ot_ps is the softmax denominator.
            # reciprocal_approx_fast ignores PSUM partition offsets, so bounce
            # the denominator rows through SBUF; the ot banks are released as
            # soon as the den + ot_sb copies drain (muls then read SBUF only).
            den = nrm.tile([1, 2 * CH], F32, tag="den", name=f"den{ch}_{t}")
            rden = nrm.tile([1, 2 * CH], F32, tag="rden", name=f"rden{ch}_{t}")
            rden_bf = nrm.tile([1, 2 * CH], BF16, tag="rdenb", name=f"rdenb{ch}_{t}")
            ot_sb = nrm.tile([128, CH], BF16, tag="otsb", name=f"ot_sb{ch}_{t}")
            nc.vector.tensor_copy(den[0:1, 0:CH], ot_ps[0][64:65, :])
            nc.vector.tensor_copy(den[0:1, CH:2 * CH], ot_ps[1][64:65, :])
            for hh in range(2):
                nc.vector.tensor_copy(ot_sb[64 * hh:64 * hh + 64, :], ot_ps[hh][0:64, :])
            nc.vector.reciprocal_approx_fast(rden[:], den[:])
            nc.vector.tensor_copy(rden_bf[:], rden[:])
            for hh in range(2):
                rbc_ps = ps_mm.tile([64, CH], F32, tag="ps", name=f"rbc{ch}_{t}_{hh}")
                nc.tensor.matmul(
                    rbc_ps[:],
                    mk_sb[0:1, 128:192],
                    rden_bf[0:1, hh * CH:(hh + 1) * CH],
                    start=True, stop=True,
                )
                rbc_sb = nrm.tile([128, CH], BF16, tag="rbc", name=f"rbc_sb{ch}_{t}_{hh}", bufs=2)
                nc.vector.tensor_copy(rbc_sb[64 * hh:64 * hh + 64, :], rbc_ps[:])
                dst = OT[t][64 * hh: 64 * hh + 64, q0:q0 + CH]
                nc.vector.tensor_mul(dst, ot_sb[64 * hh:64 * hh + 64, :], rbc_sb[64 * hh:64 * hh + 64, :])
                if has_bv:
                    nc.vector.tensor_scalar_add(dst, dst, bv_sb[64 * hh:64 * hh + 64, t:t + 1])
            # ship this pair's 128 OT rows for the quad exchange
            nc.sync.dma_start(ag_ins[ch][128 * t:128 * (t + 1), :], OT[t][:, q0:q0 + CH])
            if t == NRT - 1:
                nc.gpsimd.collective_compute(
                    "AllGather",
                    mybir.AluOpType.bypass,
                    replica_groups=[[0, 1, 2, 3], [4, 5, 6, 7]],
                    ins=[ag_ins[ch].opt()],
                    outs=[ag_outs[ch].opt()],
                )

        def cproj_chunk(ch):
            g_sb = cpj.tile([128, NK * 128], BF16, tag="g", name=f"g{ch}")
            base = ag_outs[ch].rearrange("(k p) c -> p k c", k=NK)[:, :, 0:128]
            nc.sync.dma_start(
                g_sb.rearrange("p (k c) -> p k c", k=NK),
                AP(base.tensor, goff, base.ap, dep_tracking_offset=0),
            )
            for n in range(2):
                po = ps_mm.tile([128, CH], F32, tag="ps", name=f"po{ch}_{n}")
                for kt in range(NK):
                    nc.tensor.matmul(
                        po[:],
                        g_sb[:, kt * 128:(kt + 1) * 128],
                        wp_sb[:, kt * D + n * CH: kt * D + (n + 1) * CH],
                        start=(kt == 0), stop=(kt == NK - 1),
                    )
                ob = outp.tile([128, CH], F32, tag="ob", name=f"ob{ch}_{n}")
                if has_bp:
                    nc.vector.tensor_add(ob[:], po[:], bp_sb[:, n * CH:(n + 1) * CH])
                else:
                    nc.vector.tensor_copy(ob[:], po[:])
                nc.sync.dma_start(out_d[ch * 128:(ch + 1) * 128, n * CH:(n + 1) * CH], ob[:])

        # attention order 1, 2, 3, 0: the final exposed AllGather+c_proj tail
        # belongs to the smallest chunk (0). Each attention chunk weaves the
        # next chunk's QKV production (and the previous chunk's c_proj) into
        # its k-tile loop as PE filler.
        import itertools
        drain(qkt_units(0))
        drain(v_units(0))
        drain(qkt_units(1))
        # fill rates are chosen so V-strip production always outpaces the
        # attention loop's own consumption of those strips (5 units/strip,
        # strip kt is read at step kt): rate*k >= 5*(k-4*ch+1) for all k
        f1 = itertools.chain(v_units(1), qkt_units(2))
        att_pair(1, 0, f1, 4)
        att_pair(1, 1, f1, 4)
        drain(f1)
        f2 = itertools.chain(v_units(2), qkt_units(3))
        att_pair(2, 0, f2, 4)
        att_pair(2, 1, f2, 4)
        drain(f2)
        cproj_chunk(1)
        f3 = v_units(3)
        att_pair(3, 0, f3, 2)
        att_pair(3, 1, f3, 2)
        drain(f3)
        cproj_chunk(2)
        att_pair(0, 0)
        att_pair(0, 1)
        cproj_chunk(3)
        cproj_chunk(0)
        if dump_ot:
            for t in range(NRT):
                dbg_f32 = outp.tile([128, S], F32, tag="dbgf", name=f"dbgf{t}")
                nc.vector.tensor_copy(dbg_f32[:], OT[t][:])
                nc.sync.dma_start(dbg_d[:, t * S:(t + 1) * S], dbg_f32[:])

    nc.compile()
    return nc


_prog_cache = {}


def _get_prog(has_bv, has_bp, has_bqk):
    key = (has_bv, has_bp, has_bqk)
    if key not in _prog_cache:
        _prog_cache[key] = _build(has_bv, has_bp, has_bqk)
    return _prog_cache[key]


def _interleave(w, cols):
    # [D, cols] -> [128, NK*cols] with k-tile kt at column block kt
    return np.ascontiguousarray(
        w.reshape(NK, 128, cols).transpose(1, 0, 2).reshape(128, NK * cols)
    )


def _prepare(x, w_attn, b_attn, w_proj, b_proj):
    x = np.asarray(x, dtype=np.float32)
    w_attn = np.asarray(w_attn, dtype=np.float32)
    b_attn = np.asarray(b_attn, dtype=np.float32)
    w_proj = np.asarray(w_proj, dtype=np.float32)
    b_proj = np.asarray(b_proj, dtype=np.float32)

    has_bv = bool(np.any(b_attn[2 * D:]))
    has_bp = bool(np.any(b_proj))
    has_bqk = bool(np.any(b_attn[:2 * D]))
    nc = _get_prog(has_bv, has_bp, has_bqk)

    ii = np.arange(128)[:, None]
    jj = np.arange(128)[None, :]
    masks = np.zeros((128, 512), dtype=np.float32)
    masks[:, :128] = np.where(jj >= ii, 0.0, -1e9)  # additive causal mask
    masks[0, 128:192] = 1.0   # ones row (base partition 0) for rden broadcast
    masks[:, 256:384] = (ii == jj)  # identity for the mask accumulate-matmul
    masks_bf = masks.astype(ml_dtypes.bfloat16)

    wp_il = _interleave(w_proj, D).astype(ml_dtypes.bfloat16)

    in_maps = []
    for c in range(N_CORES):
        b, g = divmod(c, 4)
        q0 = g * DG
        k0 = D + g * DG
        v0 = 2 * D + g * DG
        wv_ext = np.zeros((D, VW), dtype=np.float32)
        for hl in range(HG):
            wv_ext[:, hl * (HD + 1):hl * (HD + 1) + HD] = w_attn[:, v0 + hl * HD: v0 + (hl + 1) * HD]
        xt = x[b].T  # [D, S]
        xt_il = np.ascontiguousarray(
            xt.reshape(NK, 128, NCH, CH).transpose(1, 2, 0, 3).reshape(128, NCH * NK * CH)
        ).astype(ml_dtypes.bfloat16)
        in_maps.append({
            "xt": xt_il,
            "wq": _interleave(w_attn[:, q0:q0 + DG], DG).astype(ml_dtypes.bfloat16),
            "wk": _interleave(w_attn[:, k0:k0 + DG], DG).astype(ml_dtypes.bfloat16),
            "wv": _interleave(wv_ext, VW).astype(ml_dtypes.bfloat16),
            "wp": wp_il,
            "bq": np.ascontiguousarray(b_attn[q0:q0 + DG].reshape(NRT, 128).T),
            "bk": np.ascontiguousarray(b_attn[k0:k0 + DG].reshape(NRT, 128).T),
            "bv": np.ascontiguousarray(b_attn[v0:v0 + DG].reshape(NRT, 128).T),
            "bp": np.broadcast_to(b_proj, (128, D)).astype(np.float32).copy(),
            "masks": masks_bf,
        })
    return nc, in_maps


def _assemble(results):
    out = np.empty((B, S, D), dtype=np.float32)
    for c in range(N_CORES):
        b, g = divmod(c, 4)
        o = results[c]["out"]
        for ch in range(NCH):
            tok = ch * CH + g * 128
            out[b, tok:tok + 128, :] = o[ch * 128:(ch + 1) * 128, :]
    return out


def kernel(x, w_attn, b_attn, w_proj, b_proj):
    nc, in_maps = _prepare(x, w_attn, b_attn, w_proj, b_proj)
    res = run_bass_kernel_spmd(nc, in_maps, list(range(N_CORES)))
    return _assemble(res.results)


# revision 29
# speedup vs baseline: 1.0855x; 1.0855x over previous
"""GPT2 eager causal attention (B=2, S=2048, D=1024, H=16, HD=64) on 8 TRN2 NeuronCores.

Sharding (data + head/tensor parallel): core c -> (batch b = c//4, head-group
g = c%4) -- 4 heads per group; each quad (same batch) exchanges attention
outputs with a small bf16 AllToAll and computes the full c_proj locally for
its own token slices (no ReduceScatter, no fp32 partials).

Per-core pipeline:
  0. host pre-transposes x -> xT[d, s] and pre-interleaves every weight into
     its SBUF layout, so all device DMAs are large and contiguous.
  1. per 512-token chunk ch: QT/KT row-tiles and V strips via PE matmuls
     (d-contraction over 8 k-tiles), interleaved with...
  2. attention for chunk ch, head-pair t: score tiles for both heads of the
     pair go into one [128,1024] 2-bank PSUM tile via two row-group-packed
     K=64 matmuls (rows 0:64 / 64:128 -> concurrent on the PE sub-arrays);
     ONE exp covers both heads; diagonal k-tiles exp only the causal suffix
     and the st/ot matmuls skip the masked prefix columns entirely.
     V carries a ones-column per head so ot row 64 is the softmax denominator;
     reciprocal straight from PSUM, broadcast to 128 partitions with a K=2
     sel-matmul, normalize on DVE.
  3. OT chunk [256, 512]bf16 -> DRAM -> AllToAll over the quad (each core
     receives the full-model OT columns for its 128-token slice) -> local
     c_proj with the full w_proj -> fp32 out rows.
"""
from contextlib import ExitStack

import ml_dtypes
import numpy as np

import concourse.bacc as bacc
import concourse.mybir as mybir
import concourse.tile as tile
from concourse.bass import AP
from concourse.bass_utils import run_bass_kernel_spmd

F32 = mybir.dt.float32
BF16 = mybir.dt.bfloat16

B, S, D, H, HD = 2, 2048, 1024, 16, 64
N_CORES = 8
HG = 4               # heads per group (per core)
DG = HG * HD         # 256 q/k channels per group
VW = HG * (HD + 1)   # 260: 64 v-cols + 1 ones-col per head
NK = D // 128        # 8 contraction tiles over d
CH = 512             # q-chunk (one PSUM bank of fp32)
NCH = S // CH        # 4
NRT = DG // 128      # 2 channel row-tiles per group
WARMUP_MM = 110       # dummy matmuls to lift the PE HAM throttle before real work


def _build(has_bv: bool, has_bp: bool, has_bqk: bool = False, dump_ot: bool = False):
    nc = bacc.Bacc("TRN2", target_bir_lowering=False, debug=False, num_devices=N_CORES)
    dbg_d = nc.dram_tensor("dbg", [128, NRT * S], F32, kind="ExternalOutput").ap() if dump_ot else None

    xt_d = nc.dram_tensor("xt", [128, NCH * NK * CH], BF16, kind="ExternalInput").ap()
    wq_d = nc.dram_tensor("wq", [128, NK * DG], BF16, kind="ExternalInput").ap()
    wk_d = nc.dram_tensor("wk", [128, NK * DG], BF16, kind="ExternalInput").ap()
    wv_d = nc.dram_tensor("wv", [128, NK * VW], BF16, kind="ExternalInput").ap()
    wp_d = nc.dram_tensor("wp", [128, NK * D], BF16, kind="ExternalInput").ap()
    bq_d = nc.dram_tensor("bq", [128, NRT], F32, kind="ExternalInput").ap()
    bk_d = nc.dram_tensor("bk", [128, NRT], F32, kind="ExternalInput").ap()
    bv_d = nc.dram_tensor("bv", [128, NRT], F32, kind="ExternalInput").ap()
    bp_d = nc.dram_tensor("bp", [128, D], F32, kind="ExternalInput").ap()
    mk_d = nc.dram_tensor("masks", [128, 512], BF16, kind="ExternalInput").ap()
    out_d = nc.dram_tensor("out", [CH, D], F32, kind="ExternalOutput").ap()

    EXP = mybir.ActivationFunctionType.Exp
    IDENT = mybir.ActivationFunctionType.Identity

    with ExitStack() as ctx:
        tc = ctx.enter_context(tile.TileContext(nc))
        wpool = ctx.enter_context(tc.tile_pool(name="w", bufs=1))
        big = ctx.enter_context(tc.tile_pool(name="big", bufs=1))
        qkvp = ctx.enter_context(tc.tile_pool(name="qkv", bufs=1))
        stp = ctx.enter_context(tc.tile_pool(name="stx", bufs=4))
        nrm = ctx.enter_context(tc.tile_pool(name="nrm", bufs=2))
        cpj = ctx.enter_context(tc.tile_pool(name="cpj", bufs=2))
        outp = ctx.enter_context(tc.tile_pool(name="outp", bufs=3))
        ps_mm = ctx.enter_context(tc.tile_pool(name="psmm", bufs=2, space="PSUM"))
        ps_st = ctx.enter_context(tc.tile_pool(name="psst", bufs=2, space="PSUM"))
        ps_ot = ctx.enter_context(tc.tile_pool(name="psot", bufs=1, space="PSUM"))
        dram = ctx.enter_context(tc.tile_pool(name="dram", bufs=1, space="DRAM"))

        # ---- tiny constants first so the PE warm-up can start immediately
        # mk_sb cols 0:128 = triangular causal mask; rows 0:2 cols 128:256 =
        # the sel pattern that broadcasts rden row 0 -> partitions 0:64 and
        # row 1 -> partitions 64:128 via a K=2 matmul.
        mk_sb = wpool.tile([128, 512], BF16)
        nc.sync.dma_start(mk_sb[:], mk_d[:])
        if WARMUP_MM:
            warm_ps = ps_mm.tile([128, 512], F32, tag="ps", name="warm_ps")
            for i in range(WARMUP_MM):
                nc.tensor.matmul(
                    warm_ps[:, 0:256], mk_sb[:, 256:384], mk_sb[:, 0:256],
                    start=True, stop=True,
                )

        # ---- weights / x strips (host already laid out in SBUF order)
        xt_sb = big.tile([128, NCH * NK * CH], BF16, name="xt_sb")
        xt3 = xt_sb.rearrange("p (k c) -> p k c", k=NK)
        wq_sb = wpool.tile([128, NK * DG], BF16)
        wk_sb = wpool.tile([128, NK * DG], BF16)
        wv_sb = wpool.tile([128, NK * VW], BF16)
        wp_sb = wpool.tile([128, NK * D], BF16)
        bq_sb = wpool.tile([128, NRT], F32) if has_bqk else None
        bk_sb = wpool.tile([128, NRT], F32) if has_bqk else None
        bv_sb = wpool.tile([128, NRT], F32) if has_bv else None
        bp_sb = wpool.tile([128, D], F32) if has_bp else None

        def load_x_chunk(ch, eng):
            # 4 sub-DMAs so the chunk spreads across parallel DMA queues
            for q in range(4):
                eng.dma_start(
                    xt3[:, 2 * q:2 * q + 2, ch * CH:(ch + 1) * CH],
                    xt_d[:, ch * NK * CH + 2 * q * CH: ch * NK * CH + (2 * q + 2) * CH],
                )

        # sync HWDGE carries chunk 0 + qkv weights; the scalar HWDGE queue
        # (idle until the first exp) carries the rest -> 2x prologue bandwidth
        load_x_chunk(0, nc.sync)
        nc.sync.dma_start(wk_sb[:], wk_d[:])
        nc.sync.dma_start(wq_sb[:], wq_d[:])
        nc.sync.dma_start(wv_sb[:], wv_d[:])
        for ch in range(1, NCH):
            load_x_chunk(ch, nc.scalar)
        nc.scalar.dma_start(wp_sb[:, :NK * D // 2], wp_d[:, :NK * D // 2])
        nc.scalar.dma_start(wp_sb[:, NK * D // 2:], wp_d[:, NK * D // 2:])
        if has_bqk:
            nc.sync.dma_start(bq_sb[:], bq_d[:])
            nc.sync.dma_start(bk_sb[:], bk_d[:])
        if has_bv:
            nc.sync.dma_start(bv_sb[:], bv_d[:])
        if has_bp:
            nc.sync.dma_start(bp_sb[:], bp_d[:])

        # ---- persistent SBUF tensors
        QT = [qkvp.tile([128, S], BF16, name=f"qT{rt}") for rt in range(NRT)]
        KT = [qkvp.tile([128, S], BF16, name=f"kT{rt}") for rt in range(NRT)]
        V = [qkvp.tile([128, VW], BF16, tag=f"v{st}", name=f"v{st}") for st in range(S // 128)]
        OT = [big.tile([128, S], BF16, name=f"OT{t}") for t in range(NRT)]

        def qkt_units(ch):
            """QT/KT row-tiles for chunk ch as a generator of small PE bursts."""
            for store, w_sb, b_sb, nm in ((KT, wk_sb, bk_sb, "k"), (QT, wq_sb, bq_sb, "q")):
                for rt in range(NRT):
                    ps = ps_mm.tile([128, CH], F32, tag="ps", name=f"ps{nm}{rt}_{ch}")
                    for kt in range(NK):
                        nc.tensor.matmul(
                            ps[:],
                            w_sb[:, kt * DG + rt * 128: kt * DG + (rt + 1) * 128],
                            xt3[:, kt, ch * CH:(ch + 1) * CH],
                            start=(kt == 0), stop=(kt == NK - 1),
                        )
                        if kt % 2 == 1:
                            yield
                    dst = store[rt][:, ch * CH:(ch + 1) * CH]
                    if has_bqk:
                        nc.scalar.activation(dst, ps[:], IDENT, bias=b_sb[:, rt:rt + 1])
                    else:
                        # scalar engine: it idles during qkv production and
                        # this keeps the DVE free for the normalize chain
                        nc.scalar.copy(dst, ps[:])
                    yield

        def v_units(ch):
            """V strips for chunk ch as a generator of small PE bursts."""
            for st in range(4 * ch, 4 * ch + 4):
                ps = ps_mm.tile([128, CH], F32, tag="ps", name=f"psv{st}")
                for kt in range(NK):
                    nc.tensor.matmul(
                        ps[:, :VW],
                        xt3[:, kt, st * 128:(st + 1) * 128],
                        wv_sb[:, kt * VW:(kt + 1) * VW],
                        start=(kt == 0), stop=(kt == NK - 1),
                    )
                    if kt % 2 == 1:
                        yield
                vt = V[st]
                nc.vector.tensor_copy(vt[:], ps[:, :VW])
                for hl in range(HG):
                    ones_col = vt[:, hl * (HD + 1) + HD: (hl + 1) * (HD + 1)].bitcast(mybir.dt.uint16)
                    nc.vector.memset(ones_col, 0x3F80)  # bf16 1.0
                yield

        def drain(gen):
            for _ in gen:
                pass

        # DRAM staging for the per-chunk AllGather: each core contributes its
        # [256, 512] bf16 OT chunk and receives the full-model [1024, 512];
        # it then loads only the 128-token column slice it owns (dynamic
        # offset from the device id).
        ag_ins, ag_outs = [], []
        for ch in range(NCH):
            ai = dram.tile([DG, CH], BF16, tag=f"agi{ch}", name=f"ag_in{ch}")
            ao = dram.tile([HG * DG, CH], BF16, tag=f"ago{ch}", name=f"ag_out{ch}")
            ag_ins.append(ai)
            ag_outs.append(ao)
        pid = nc.sync.partition_id()
        goff = (pid % HG) * 128  # my token-block column offset in the gathered chunk

        def att_pair(ch, t, filler=None, fill_rate=2):
            """Attention for chunk ch, head pair t (heads 2t, 2t+1).
            After each k-tile step, pulls up to fill_rate units of independent
            PE work from `filler` -- these absorb the PE's exp-wait stalls so
            the HAM throttle never re-engages mid-attention."""
            q0 = ch * CH
            nkt = 4 * (ch + 1)
            ot_ps = {}
            ot_ps[0] = ps_ot.tile([HD + 1, CH], F32, tag="otA", name=f"otA{ch}_{t}")
            ot_ps[1] = ps_ot.tile([HD + 1, CH], F32, tag="otB", name=f"otB{ch}_{t}")
            for kt in range(nkt):
                d = kt - 4 * ch
                d0 = max(d, 0) * 128
                st_ps = ps_st.tile([128, 2 * CH], F32, tag="st", name=f"st{ch}_{t}_{kt}")
                st_sb = stp.tile([128, 2 * CH], BF16, tag="stsb", name=f"se{ch}_{t}_{kt}")
                diag = d >= 0
                for hh in range(2):  # row-group packed pair: concurrent on PE
                    off = 64 * hh
                    nc.tensor.matmul(
                        st_ps[:, hh * CH + d0: (hh + 1) * CH],
                        KT[t][off:off + 64, kt * 128:(kt + 1) * 128],
                        QT[t][off:off + 64, q0 + d0: q0 + CH],
                        start=True, stop=not diag,
                    )
                if diag:
                    # causal mask: PE-accumulate -1e9 onto the diagonal block
                    # (identity @ maskneg), so exp gives exact zeros and the
                    # DVE stays out of the exp->ot chain
                    for hh in range(2):
                        nc.tensor.matmul(
                            st_ps[:, hh * CH + d0: hh * CH + d0 + 128],
                            mk_sb[:, 256:384],
                            mk_sb[:, 0:128],
                            start=False, stop=True,
                        )
                if d <= 0:
                    # one exp covers both heads' 512-column halves
                    nc.scalar.activation(st_sb[:], st_ps[:], EXP, scale=0.125)
                else:
                    for hh in range(2):
                        nc.scalar.activation(
                            st_sb[:, hh * CH + d0: (hh + 1) * CH],
                            st_ps[:, hh * CH + d0: (hh + 1) * CH],
                            EXP, scale=0.125,
                        )
                for hh in range(2):
                    hl = 2 * t + hh
                    nc.tensor.matmul(
                        ot_ps[hh][:, d0:],
                        V[kt][:, hl * (HD + 1):(hl + 1) * (HD + 1)],
                        st_sb[:, hh * CH + d0:(hh + 1) * CH],
                        start=(kt == 0), stop=(kt == nkt - 1),
                    )
                if filler is not None:
                    for _ in range(fill_rate):
                        if next(filler, StopIteration) is StopIteration:
                            filler = None
                            break
            # normalize: row 64 of each ot_ps is the softmax denominator.
            # reciprocal_approx_fast ignores PSUM partition offsets, so bounce
            # the denominator rows through SBUF; the ot banks are released as
            # soon as the den + ot_sb copies drain (muls then read SBUF only).
            den = nrm.tile([1, 2 * CH], F32, tag="den", name=f"den{ch}_{t}")
            rden = nrm.tile([1, 2 * CH], F32, tag="rden", name=f"rden{ch}_{t}")
            rden_bf = nrm.tile([1, 2 * CH], BF16, tag="rdenb", name=f"rdenb{ch}_{t}")
            ot_sb = nrm.tile([128, CH], BF16, tag="otsb", name=f"ot_sb{ch}_{t}")
            nc.vector.tensor_copy(den[0:1, 0:CH], ot_ps[0][64:65, :])
            nc.vector.tensor_copy(den[0:1, CH:2 * CH], ot_ps[1][64:65, :])
            for hh in range(2):
                nc.vector.tensor_copy(ot_sb[64 * hh:64 * hh + 64, :], ot_ps[hh][0:64, :])
            nc.vector.reciprocal_approx_fast(rden[:], den[:])
            nc.vector.tensor_copy(rden_bf[:], rden[:])
            for hh in range(2):
                rbc_ps = ps_mm.tile([64, CH], F32, tag="ps", name=f"rbc{ch}_{t}_{hh}")
                nc.tensor.matmul(
                    rbc_ps[:],
                    mk_sb[0:1, 128:192],
                    rden_bf[0:1, hh * CH:(hh + 1) * CH],
                    start=True, stop=True,
                )
                rbc_sb = nrm.tile([128, CH], BF16, tag="rbc", name=f"rbc_sb{ch}_{t}_{hh}", bufs=2)
                nc.vector.tensor_copy(rbc_sb[64 * hh:64 * hh + 64, :], rbc_ps[:])
                dst = OT[t][64 * hh: 64 * hh + 64, q0:q0 + CH]
                nc.vector.tensor_mul(dst, ot_sb[64 * hh:64 * hh + 64, :], rbc_sb[64 * hh:64 * hh + 64, :])
                if has_bv:
                    nc.vector.tensor_scalar_add(dst, dst, bv_sb[64 * hh:64 * hh + 64, t:t + 1])
            # ship this pair's 128 OT rows for the quad exchange
            nc.sync.dma_start(ag_ins[ch][128 * t:128 * (t + 1), :], OT[t][:, q0:q0 + CH])
            if t == NRT - 1:
                nc.gpsimd.collective_compute(
                    "AllGather",
                    mybir.AluOpType.bypass,
                    replica_groups=[[0, 1, 2, 3], [4, 5, 6, 7]],
                    ins=[ag_ins[ch].opt()],
                    outs=[ag_outs[ch].opt()],
                )

        def cproj_chunk(ch):
            g_sb = cpj.tile([128, NK * 128], BF16, tag="g", name=f"g{ch}")
            base = ag_outs[ch].rearrange("(k p) c -> p k c", k=NK)[:, :, 0:128]
            nc.sync.dma_start(
                g_sb.rearrange("p (k c) -> p k c", k=NK),
                AP(base.tensor, goff, base.ap, dep_tracking_offset=0),
            )
            for n in range(2):
                po = ps_mm.tile([128, CH], F32, tag="ps", name=f"po{ch}_{n}")
                for kt in range(NK):
                    nc.tensor.matmul(
                        po[:],
                        g_sb[:, kt * 128:(kt + 1) * 128],
                        wp_sb[:, kt * D + n * CH: kt * D + (n + 1) * CH],
                        start=(kt == 0), stop=(kt == NK - 1),
                    )
                ob = outp.tile([128, CH], F32, tag="ob", name=f"ob{ch}_{n}")
                if has_bp:
                    nc.vector.tensor_add(ob[:], po[:], bp_sb[:, n * CH:(n + 1) * CH])
                else:
                    nc.vector.tensor_copy(ob[:], po[:])
                nc.sync.dma_start(out_d[ch * 128:(ch + 1) * 128, n * CH:(n + 1) * CH], ob[:])

        # attention order 1, 2, 3, 0: the final exposed AllGather+c_proj tail
        # belongs to the smallest chunk (0). Each attention chunk weaves the
        # next chunk's QKV production (and the previous chunk's c_proj) into
        # its k-tile loop as PE filler.
        import itertools
        drain(qkt_units(0))
        drain(v_units(0))
        drain(qkt_units(1))
        # fill rates are chosen so V-strip production always outpaces the
        # attention loop's own consumption of those strips (5 units/strip,
        # strip kt is read at step kt): rate*k >= 5*(k-4*ch+1) for all k
        f1 = itertools.chain(v_units(1), qkt_units(2))
        att_pair(1, 0, f1, 3)
        att_pair(1, 1, f1, 3)
        drain(f1)
        f2 = itertools.chain(v_units(2), qkt_units(3))
        att_pair(2, 0, f2, 2)
        att_pair(2, 1, f2, 2)
        drain(f2)
        cproj_chunk(1)
        f3 = v_units(3)
        att_pair(3, 0, f3, 2)
        att_pair(3, 1, f3, 2)
        drain(f3)
        cproj_chunk(2)
        att_pair(0, 0)
        att_pair(0, 1)
        cproj_chunk(3)
        cproj_chunk(0)
        if dump_ot:
            for t in range(NRT):
                dbg_f32 = outp.tile([128, S], F32, tag="dbgf", name=f"dbgf{t}")
                nc.vector.tensor_copy(dbg_f32[:], OT[t][:])
                nc.sync.dma_start(dbg_d[:, t * S:(t + 1) * S], dbg_f32[:])

    nc.compile()
    return nc


_prog_cache = {}


def _get_prog(has_bv, has_bp, has_bqk):
    key = (has_bv, has_bp, has_bqk)
    if key not in _prog_cache:
        _prog_cache[key] = _build(has_bv, has_bp, has_bqk)
    return _prog_cache[key]


def _interleave(w, cols):
    # [D, cols] -> [128, NK*cols] with k-tile kt at column block kt
    return np.ascontiguousarray(
        w.reshape(NK, 128, cols).transpose(1, 0, 2).reshape(128, NK * cols)
    )


def _prepare(x, w_attn, b_attn, w_proj, b_proj):
    x = np.asarray(x, dtype=np.float32)
    w_attn = np.asarray(w_attn, dtype=np.float32)
    b_attn = np.asarray(b_attn, dtype=np.float32)
    w_proj = np.asarray(w_proj, dtype=np.float32)
    b_proj = np.asarray(b_proj, dtype=np.float32)

    has_bv = bool(np.any(b_attn[2 * D:]))
    has_bp = bool(np.any(b_proj))
    has_bqk = bool(np.any(b_attn[:2 * D]))
    nc = _get_prog(has_bv, has_bp, has_bqk)

    ii = np.arange(128)[:, None]
    jj = np.arange(128)[None, :]
    masks = np.zeros((128, 512), dtype=np.float32)
    masks[:, :128] = np.where(jj >= ii, 0.0, -1e9)  # additive causal mask
    masks[0, 128:192] = 1.0   # ones row (base partition 0) for rden broadcast
    masks[:, 256:384] = (ii == jj)  # identity for the mask accumulate-matmul
    masks_bf = masks.astype(ml_dtypes.bfloat16)

    wp_il = _interleave(w_proj, D).astype(ml_dtypes.bfloat16)

    in_maps = []
    for c in range(N_CORES):
        b, g = divmod(c, 4)
        q0 = g * DG
        k0 = D + g * DG
        v0 = 2 * D + g * DG
        wv_ext = np.zeros((D, VW), dtype=np.float32)
        for hl in range(HG):
            wv_ext[:, hl * (HD + 1):hl * (HD + 1) + HD] = w_attn[:, v0 + hl * HD: v0 + (hl + 1) * HD]
        xt = x[b].T  # [D, S]
        xt_il = np.ascontiguousarray(
            xt.reshape(NK, 128, NCH, CH).transpose(1, 2, 0, 3).reshape(128, NCH * NK * CH)
        ).astype(ml_dtypes.bfloat16)
        in_maps.append({
            "xt": xt_il,
            "wq": _interleave(w_attn[:, q0:q0 + DG], DG).astype(ml_dtypes.bfloat16),
            "wk": _interleave(w_attn[:, k0:k0 + DG], DG).astype(ml_dtypes.bfloat16),
            "wv": _interleave(wv_ext, VW).astype(ml_dtypes.bfloat16),
            "wp": wp_il,
            "bq": np.ascontiguousarray(b_attn[q0:q0 + DG].reshape(NRT, 128).T),
            "bk": np.ascontiguousarray(b_attn[k0:k0 + DG].reshape(NRT, 128).T),
            "bv": np.ascontiguousarray(b_attn[v0:v0 + DG].reshape(NRT, 128).T),
            "bp": np.broadcast_to(b_proj, (128, D)).astype(np.float32).copy(),
            "masks": masks_bf,
        })
    return nc, in_maps


def _assemble(results):
    out = np.empty((B, S, D), dtype=np.float32)
    for c in range(N_CORES):
        b, g = divmod(c, 4)
        o = results[c]["out"]
        for ch in range(NCH):
            tok = ch * CH + g * 128
            out[b, tok:tok + 128, :] = o[ch * 128:(ch + 1) * 128, :]
    return out


def kernel(x, w_attn, b_attn, w_proj, b_proj):
    nc, in_maps = _prepare(x, w_attn, b_attn, w_proj, b_proj)
    res = run_bass_kernel_spmd(nc, in_maps, list(range(N_CORES)))
    return _assemble(res.results)


# revision 30
# speedup vs baseline: 1.1007x; 1.0139x over previous
"""GPT2 eager causal attention (B=2, S=2048, D=1024, H=16, HD=64) on 8 TRN2 NeuronCores.

Sharding (data + head/tensor parallel): core c -> (batch b = c//4, head-group
g = c%4) -- 4 heads per group; each quad (same batch) exchanges attention
outputs with a small bf16 AllToAll and computes the full c_proj locally for
its own token slices (no ReduceScatter, no fp32 partials).

Per-core pipeline:
  0. host pre-transposes x -> xT[d, s] and pre-interleaves every weight into
     its SBUF layout, so all device DMAs are large and contiguous.
  1. per 512-token chunk ch: QT/KT row-tiles and V strips via PE matmuls
     (d-contraction over 8 k-tiles), interleaved with...
  2. attention for chunk ch, head-pair t: score tiles for both heads of the
     pair go into one [128,1024] 2-bank PSUM tile via two row-group-packed
     K=64 matmuls (rows 0:64 / 64:128 -> concurrent on the PE sub-arrays);
     ONE exp covers both heads; diagonal k-tiles exp only the causal suffix
     and the st/ot matmuls skip the masked prefix columns entirely.
     V carries a ones-column per head so ot row 64 is the softmax denominator;
     reciprocal straight from PSUM, broadcast to 128 partitions with a K=2
     sel-matmul, normalize on DVE.
  3. OT chunk [256, 512]bf16 -> DRAM -> AllToAll over the quad (each core
     receives the full-model OT columns for its 128-token slice) -> local
     c_proj with the full w_proj -> fp32 out rows.
"""
from contextlib import ExitStack

import ml_dtypes
import numpy as np

import concourse.bacc as bacc
import concourse.mybir as mybir
import concourse.tile as tile
from concourse.bass import AP
from concourse.bass_utils import run_bass_kernel_spmd

F32 = mybir.dt.float32
BF16 = mybir.dt.bfloat16

B, S, D, H, HD = 2, 2048, 1024, 16, 64
N_CORES = 8
HG = 4               # heads per group (per core)
DG = HG * HD         # 256 q/k channels per group
VW = HG * (HD + 1)   # 260: 64 v-cols + 1 ones-col per head
NK = D // 128        # 8 contraction tiles over d
CH = 512             # q-chunk (one PSUM bank of fp32)
NCH = S // CH        # 4
NRT = DG // 128      # 2 channel row-tiles per group
WARMUP_MM = 110       # dummy matmuls to lift the PE HAM throttle before real work


def _build(has_bv: bool, has_bp: bool, has_bqk: bool = False, dump_ot: bool = False):
    nc = bacc.Bacc("TRN2", target_bir_lowering=False, debug=False, num_devices=N_CORES)
    dbg_d = nc.dram_tensor("dbg", [128, NRT * S], F32, kind="ExternalOutput").ap() if dump_ot else None

    xt_d = nc.dram_tensor("xt", [128, NCH * NK * CH], BF16, kind="ExternalInput").ap()
    wq_d = nc.dram_tensor("wq", [128, NK * DG], BF16, kind="ExternalInput").ap()
    wk_d = nc.dram_tensor("wk", [128, NK * DG], BF16, kind="ExternalInput").ap()
    wv_d = nc.dram_tensor("wv", [128, NK * VW], BF16, kind="ExternalInput").ap()
    wp_d = nc.dram_tensor("wp", [128, NK * D], BF16, kind="ExternalInput").ap()
    bq_d = nc.dram_tensor("bq", [128, NRT], F32, kind="ExternalInput").ap()
    bk_d = nc.dram_tensor("bk", [128, NRT], F32, kind="ExternalInput").ap()
    bv_d = nc.dram_tensor("bv", [128, NRT], F32, kind="ExternalInput").ap()
    bp_d = nc.dram_tensor("bp", [128, D], F32, kind="ExternalInput").ap()
    mk_d = nc.dram_tensor("masks", [128, 512], BF16, kind="ExternalInput").ap()
    out_d = nc.dram_tensor("out", [CH, D], F32, kind="ExternalOutput").ap()

    EXP = mybir.ActivationFunctionType.Exp
    IDENT = mybir.ActivationFunctionType.Identity

    with ExitStack() as ctx:
        tc = ctx.enter_context(tile.TileContext(nc))
        wpool = ctx.enter_context(tc.tile_pool(name="w", bufs=1))
        big = ctx.enter_context(tc.tile_pool(name="big", bufs=1))
        qkvp = ctx.enter_context(tc.tile_pool(name="qkv", bufs=1))
        stp = ctx.enter_context(tc.tile_pool(name="stx", bufs=4))
        nrm = ctx.enter_context(tc.tile_pool(name="nrm", bufs=2))
        cpj = ctx.enter_context(tc.tile_pool(name="cpj", bufs=2))
        outp = ctx.enter_context(tc.tile_pool(name="outp", bufs=3))
        ps_mm = ctx.enter_context(tc.tile_pool(name="psmm", bufs=2, space="PSUM"))
        ps_st = ctx.enter_context(tc.tile_pool(name="psst", bufs=2, space="PSUM"))
        ps_ot = ctx.enter_context(tc.tile_pool(name="psot", bufs=1, space="PSUM"))
        dram = ctx.enter_context(tc.tile_pool(name="dram", bufs=1, space="DRAM"))

        # ---- tiny constants first so the PE warm-up can start immediately
        # mk_sb cols 0:128 = triangular causal mask; rows 0:2 cols 128:256 =
        # the sel pattern that broadcasts rden row 0 -> partitions 0:64 and
        # row 1 -> partitions 64:128 via a K=2 matmul.
        mk_sb = wpool.tile([128, 512], BF16)
        nc.sync.dma_start(mk_sb[:], mk_d[:])
        if WARMUP_MM:
            warm_ps = ps_mm.tile([128, 512], F32, tag="ps", name="warm_ps")
            for i in range(WARMUP_MM):
                nc.tensor.matmul(
                    warm_ps[:, 0:256], mk_sb[:, 256:384], mk_sb[:, 0:256],
                    start=True, stop=True,
                )

        # ---- weights / x strips (host already laid out in SBUF order)
        xt_sb = big.tile([128, NCH * NK * CH], BF16, name="xt_sb")
        xt3 = xt_sb.rearrange("p (k c) -> p k c", k=NK)
        wq_sb = wpool.tile([128, NK * DG], BF16)
        wk_sb = wpool.tile([128, NK * DG], BF16)
        wv_sb = wpool.tile([128, NK * VW], BF16)
        wp_sb = wpool.tile([128, NK * D], BF16)
        bq_sb = wpool.tile([128, NRT], F32) if has_bqk else None
        bk_sb = wpool.tile([128, NRT], F32) if has_bqk else None
        bv_sb = wpool.tile([128, NRT], F32) if has_bv else None
        bp_sb = wpool.tile([128, D], F32) if has_bp else None

        def load_x_chunk(ch, eng):
            # 4 sub-DMAs so the chunk spreads across parallel DMA queues
            for q in range(4):
                eng.dma_start(
                    xt3[:, 2 * q:2 * q + 2, ch * CH:(ch + 1) * CH],
                    xt_d[:, ch * NK * CH + 2 * q * CH: ch * NK * CH + (2 * q + 2) * CH],
                )

        # sync HWDGE carries chunk 0 + qkv weights; the scalar HWDGE queue
        # (idle until the first exp) carries the rest -> 2x prologue bandwidth
        load_x_chunk(0, nc.sync)
        nc.sync.dma_start(wk_sb[:], wk_d[:])
        nc.sync.dma_start(wq_sb[:], wq_d[:])
        nc.sync.dma_start(wv_sb[:], wv_d[:])
        for ch in range(1, NCH):
            load_x_chunk(ch, nc.scalar)
        nc.scalar.dma_start(wp_sb[:, :NK * D // 2], wp_d[:, :NK * D // 2])
        nc.scalar.dma_start(wp_sb[:, NK * D // 2:], wp_d[:, NK * D // 2:])
        if has_bqk:
            nc.sync.dma_start(bq_sb[:], bq_d[:])
            nc.sync.dma_start(bk_sb[:], bk_d[:])
        if has_bv:
            nc.sync.dma_start(bv_sb[:], bv_d[:])
        if has_bp:
            nc.sync.dma_start(bp_sb[:], bp_d[:])

        # ---- persistent SBUF tensors
        QT = [qkvp.tile([128, S], BF16, name=f"qT{rt}") for rt in range(NRT)]
        KT = [qkvp.tile([128, S], BF16, name=f"kT{rt}") for rt in range(NRT)]
        V = [qkvp.tile([128, VW], BF16, tag=f"v{st}", name=f"v{st}") for st in range(S // 128)]
        OT = [big.tile([128, S], BF16, name=f"OT{t}") for t in range(NRT)]

        def qkt_units(ch):
            """QT/KT row-tiles for chunk ch as a generator of small PE bursts."""
            for store, w_sb, b_sb, nm in ((KT, wk_sb, bk_sb, "k"), (QT, wq_sb, bq_sb, "q")):
                for rt in range(NRT):
                    ps = ps_mm.tile([128, CH], F32, tag="ps", name=f"ps{nm}{rt}_{ch}")
                    for kt in range(NK):
                        nc.tensor.matmul(
                            ps[:],
                            w_sb[:, kt * DG + rt * 128: kt * DG + (rt + 1) * 128],
                            xt3[:, kt, ch * CH:(ch + 1) * CH],
                            start=(kt == 0), stop=(kt == NK - 1),
                        )
                        if kt % 2 == 1:
                            yield
                    dst = store[rt][:, ch * CH:(ch + 1) * CH]
                    if has_bqk:
                        nc.scalar.activation(dst, ps[:], IDENT, bias=b_sb[:, rt:rt + 1])
                    else:
                        # scalar engine: it idles during qkv production and
                        # this keeps the DVE free for the normalize chain
                        nc.scalar.copy(dst, ps[:])
                    yield

        def v_units(ch):
            """V strips for chunk ch as a generator of small PE bursts."""
            for st in range(4 * ch, 4 * ch + 4):
                ps = ps_mm.tile([128, CH], F32, tag="ps", name=f"psv{st}")
                for kt in range(NK):
                    nc.tensor.matmul(
                        ps[:, :VW],
                        xt3[:, kt, st * 128:(st + 1) * 128],
                        wv_sb[:, kt * VW:(kt + 1) * VW],
                        start=(kt == 0), stop=(kt == NK - 1),
                    )
                    if kt % 2 == 1:
                        yield
                vt = V[st]
                nc.vector.tensor_copy(vt[:], ps[:, :VW])
                for hl in range(HG):
                    ones_col = vt[:, hl * (HD + 1) + HD: (hl + 1) * (HD + 1)].bitcast(mybir.dt.uint16)
                    nc.vector.memset(ones_col, 0x3F80)  # bf16 1.0
                yield

        def drain(gen):
            for _ in gen:
                pass

        # DRAM staging for the per-chunk AllGather: each core contributes its
        # [256, 512] bf16 OT chunk and receives the full-model [1024, 512];
        # it then loads only the 128-token column slice it owns (dynamic
        # offset from the device id).
        ag_ins, ag_outs = [], []
        for ch in range(NCH):
            ai = dram.tile([DG, CH], BF16, tag=f"agi{ch}", name=f"ag_in{ch}")
            ao = dram.tile([HG * DG, CH], BF16, tag=f"ago{ch}", name=f"ag_out{ch}")
            ag_ins.append(ai)
            ag_outs.append(ao)
        pid = nc.sync.partition_id()
        goff = (pid % HG) * 128  # my token-block column offset in the gathered chunk

        def att_pair(ch, t, filler=None, fill_rate=2):
            """Attention for chunk ch, head pair t (heads 2t, 2t+1).
            After each k-tile step, pulls up to fill_rate units of independent
            PE work from `filler` -- these absorb the PE's exp-wait stalls so
            the HAM throttle never re-engages mid-attention."""
            q0 = ch * CH
            nkt = 4 * (ch + 1)
            ot_ps = {}
            ot_ps[0] = ps_ot.tile([HD + 1, CH], F32, tag="otA", name=f"otA{ch}_{t}")
            ot_ps[1] = ps_ot.tile([HD + 1, CH], F32, tag="otB", name=f"otB{ch}_{t}")
            for kt in range(nkt):
                d = kt - 4 * ch
                d0 = max(d, 0) * 128
                st_ps = ps_st.tile([128, 2 * CH], F32, tag="st", name=f"st{ch}_{t}_{kt}")
                st_sb = stp.tile([128, 2 * CH], BF16, tag="stsb", name=f"se{ch}_{t}_{kt}")
                diag = d >= 0
                for hh in range(2):  # row-group packed pair: concurrent on PE
                    off = 64 * hh
                    nc.tensor.matmul(
                        st_ps[:, hh * CH + d0: (hh + 1) * CH],
                        KT[t][off:off + 64, kt * 128:(kt + 1) * 128],
                        QT[t][off:off + 64, q0 + d0: q0 + CH],
                        start=True, stop=not diag,
                    )
                if diag:
                    # causal mask: PE-accumulate -1e9 onto the diagonal block
                    # (identity @ maskneg), so exp gives exact zeros and the
                    # DVE stays out of the exp->ot chain
                    for hh in range(2):
                        nc.tensor.matmul(
                            st_ps[:, hh * CH + d0: hh * CH + d0 + 128],
                            mk_sb[:, 256:384],
                            mk_sb[:, 0:128],
                            start=False, stop=True,
                        )
                if d <= 0:
                    # one exp covers both heads' 512-column halves
                    nc.scalar.activation(st_sb[:], st_ps[:], EXP, scale=0.125)
                else:
                    for hh in range(2):
                        nc.scalar.activation(
                            st_sb[:, hh * CH + d0: (hh + 1) * CH],
                            st_ps[:, hh * CH + d0: (hh + 1) * CH],
                            EXP, scale=0.125,
                        )
                # filler goes BEFORE the ot matmuls: in the PE's static order
                # it then occupies exactly the window where the ots would
                # stall waiting for this k-tile's exp
                if filler is not None:
                    for _ in range(fill_rate):
                        if next(filler, StopIteration) is StopIteration:
                            filler = None
                            break
                for hh in range(2):
                    hl = 2 * t + hh
                    nc.tensor.matmul(
                        ot_ps[hh][:, d0:],
                        V[kt][:, hl * (HD + 1):(hl + 1) * (HD + 1)],
                        st_sb[:, hh * CH + d0:(hh + 1) * CH],
                        start=(kt == 0), stop=(kt == nkt - 1),
                    )
            # normalize: row 64 of each ot_ps is the softmax denominator.
            # reciprocal_approx_fast ignores PSUM partition offsets, so bounce
            # the denominator rows through SBUF; the ot banks are released as
            # soon as the den + ot_sb copies drain (muls then read SBUF only).
            den = nrm.tile([1, 2 * CH], F32, tag="den", name=f"den{ch}_{t}")
            rden = nrm.tile([1, 2 * CH], F32, tag="rden", name=f"rden{ch}_{t}")
            rden_bf = nrm.tile([1, 2 * CH], BF16, tag="rdenb", name=f"rdenb{ch}_{t}")
            ot_sb = nrm.tile([128, CH], BF16, tag="otsb", name=f"ot_sb{ch}_{t}")
            nc.vector.tensor_copy(den[0:1, 0:CH], ot_ps[0][64:65, :])
            nc.vector.tensor_copy(den[0:1, CH:2 * CH], ot_ps[1][64:65, :])
            for hh in range(2):
                nc.vector.tensor_copy(ot_sb[64 * hh:64 * hh + 64, :], ot_ps[hh][0:64, :])
            nc.vector.reciprocal_approx_fast(rden[:], den[:])
            nc.vector.tensor_copy(rden_bf[:], rden[:])
            for hh in range(2):
                rbc_ps = ps_mm.tile([64, CH], F32, tag="ps", name=f"rbc{ch}_{t}_{hh}")
                nc.tensor.matmul(
                    rbc_ps[:],
                    mk_sb[0:1, 128:192],
                    rden_bf[0:1, hh * CH:(hh + 1) * CH],
                    start=True, stop=True,
                )
                rbc_sb = nrm.tile([128, CH], BF16, tag="rbc", name=f"rbc_sb{ch}_{t}_{hh}", bufs=2)
                nc.vector.tensor_copy(rbc_sb[64 * hh:64 * hh + 64, :], rbc_ps[:])
                dst = OT[t][64 * hh: 64 * hh + 64, q0:q0 + CH]
                nc.vector.tensor_mul(dst, ot_sb[64 * hh:64 * hh + 64, :], rbc_sb[64 * hh:64 * hh + 64, :])
                if has_bv:
                    nc.vector.tensor_scalar_add(dst, dst, bv_sb[64 * hh:64 * hh + 64, t:t + 1])
            # ship this pair's 128 OT rows for the quad exchange
            nc.sync.dma_start(ag_ins[ch][128 * t:128 * (t + 1), :], OT[t][:, q0:q0 + CH])
            if t == NRT - 1:
                nc.gpsimd.collective_compute(
                    "AllGather",
                    mybir.AluOpType.bypass,
                    replica_groups=[[0, 1, 2, 3], [4, 5, 6, 7]],
                    ins=[ag_ins[ch].opt()],
                    outs=[ag_outs[ch].opt()],
                )

        def cproj_chunk(ch):
            g_sb = cpj.tile([128, NK * 128], BF16, tag="g", name=f"g{ch}")
            base = ag_outs[ch].rearrange("(k p) c -> p k c", k=NK)[:, :, 0:128]
            nc.sync.dma_start(
                g_sb.rearrange("p (k c) -> p k c", k=NK),
                AP(base.tensor, goff, base.ap, dep_tracking_offset=0),
            )
            for n in range(2):
                po = ps_mm.tile([128, CH], F32, tag="ps", name=f"po{ch}_{n}")
                for kt in range(NK):
                    nc.tensor.matmul(
                        po[:],
                        g_sb[:, kt * 128:(kt + 1) * 128],
                        wp_sb[:, kt * D + n * CH: kt * D + (n + 1) * CH],
                        start=(kt == 0), stop=(kt == NK - 1),
                    )
                ob = outp.tile([128, CH], F32, tag="ob", name=f"ob{ch}_{n}")
                if has_bp:
                    nc.vector.tensor_add(ob[:], po[:], bp_sb[:, n * CH:(n + 1) * CH])
                else:
                    nc.vector.tensor_copy(ob[:], po[:])
                nc.sync.dma_start(out_d[ch * 128:(ch + 1) * 128, n * CH:(n + 1) * CH], ob[:])

        # attention order 1, 2, 3, 0: the final exposed AllGather+c_proj tail
        # belongs to the smallest chunk (0). Each attention chunk weaves the
        # next chunk's QKV production (and the previous chunk's c_proj) into
        # its k-tile loop as PE filler.
        import itertools
        drain(qkt_units(0))
        drain(v_units(0))
        drain(qkt_units(1))
        # fill rates are chosen so V-strip production always outpaces the
        # attention loop's own consumption of those strips (5 units/strip,
        # strip kt is read at step kt): rate*k >= 5*(k-4*ch+1) for all k
        f1 = itertools.chain(v_units(1), qkt_units(2))
        att_pair(1, 0, f1, 3)
        att_pair(1, 1, f1, 3)
        drain(f1)
        f2 = itertools.chain(v_units(2), qkt_units(3))
        att_pair(2, 0, f2, 2)
        att_pair(2, 1, f2, 2)
        drain(f2)
        cproj_chunk(1)
        f3 = v_units(3)
        att_pair(3, 0, f3, 2)
        att_pair(3, 1, f3, 2)
        drain(f3)
        cproj_chunk(2)
        att_pair(0, 0)
        att_pair(0, 1)
        cproj_chunk(3)
        cproj_chunk(0)
        if dump_ot:
            for t in range(NRT):
                dbg_f32 = outp.tile([128, S], F32, tag="dbgf", name=f"dbgf{t}")
                nc.vector.tensor_copy(dbg_f32[:], OT[t][:])
                nc.sync.dma_start(dbg_d[:, t * S:(t + 1) * S], dbg_f32[:])

    nc.compile()
    return nc


_prog_cache = {}


def _get_prog(has_bv, has_bp, has_bqk):
    key = (has_bv, has_bp, has_bqk)
    if key not in _prog_cache:
        _prog_cache[key] = _build(has_bv, has_bp, has_bqk)
    return _prog_cache[key]


def _interleave(w, cols):
    # [D, cols] -> [128, NK*cols] with k-tile kt at column block kt
    return np.ascontiguousarray(
        w.reshape(NK, 128, cols).transpose(1, 0, 2).reshape(128, NK * cols)
    )


def _prepare(x, w_attn, b_attn, w_proj, b_proj):
    x = np.asarray(x, dtype=np.float32)
    w_attn = np.asarray(w_attn, dtype=np.float32)
    b_attn = np.asarray(b_attn, dtype=np.float32)
    w_proj = np.asarray(w_proj, dtype=np.float32)
    b_proj = np.asarray(b_proj, dtype=np.float32)

    has_bv = bool(np.any(b_attn[2 * D:]))
    has_bp = bool(np.any(b_proj))
    has_bqk = bool(np.any(b_attn[:2 * D]))
    nc = _get_prog(has_bv, has_bp, has_bqk)

    ii = np.arange(128)[:, None]
    jj = np.arange(128)[None, :]
    masks = np.zeros((128, 512), dtype=np.float32)
    masks[:, :128] = np.where(jj >= ii, 0.0, -1e9)  # additive causal mask
    masks[0, 128:192] = 1.0   # ones row (base partition 0) for rden broadcast
    masks[:, 256:384] = (ii == jj)  # identity for the mask accumulate-matmul
    masks_bf = masks.astype(ml_dtypes.bfloat16)

    wp_il = _interleave(w_proj, D).astype(ml_dtypes.bfloat16)

    in_maps = []
    for c in range(N_CORES):
        b, g = divmod(c, 4)
        q0 = g * DG
        k0 = D + g * DG
        v0 = 2 * D + g * DG
        wv_ext = np.zeros((D, VW), dtype=np.float32)
        for hl in range(HG):
            wv_ext[:, hl * (HD + 1):hl * (HD + 1) + HD] = w_attn[:, v0 + hl * HD: v0 + (hl + 1) * HD]
        xt = x[b].T  # [D, S]
        xt_il = np.ascontiguousarray(
            xt.reshape(NK, 128, NCH, CH).transpose(1, 2, 0, 3).reshape(128, NCH * NK * CH)
        ).astype(ml_dtypes.bfloat16)
        in_maps.append({
            "xt": xt_il,
            "wq": _interleave(w_attn[:, q0:q0 + DG], DG).astype(ml_dtypes.bfloat16),
            "wk": _interleave(w_attn[:, k0:k0 + DG], DG).astype(ml_dtypes.bfloat16),
            "wv": _interleave(wv_ext, VW).astype(ml_dtypes.bfloat16),
            "wp": wp_il,
            "bq": np.ascontiguousarray(b_attn[q0:q0 + DG].reshape(NRT, 128).T),
            "bk": np.ascontiguousarray(b_attn[k0:k0 + DG].reshape(NRT, 128).T),
            "bv": np.ascontiguousarray(b_attn[v0:v0 + DG].reshape(NRT, 128).T),
            "bp": np.broadcast_to(b_proj, (128, D)).astype(np.float32).copy(),
            "masks": masks_bf,
        })
    return nc, in_maps


def _assemble(results):
    out = np.empty((B, S, D), dtype=np.float32)
    for c in range(N_CORES):
        b, g = divmod(c, 4)
        o = results[c]["out"]
        for ch in range(NCH):
            tok = ch * CH + g * 128
            out[b, tok:tok + 128, :] = o[ch * 128:(ch + 1) * 128, :]
    return out


def kernel(x, w_attn, b_attn, w_proj, b_proj):
    nc, in_maps = _prepare(x, w_attn, b_attn, w_proj, b_proj)
    res = run_bass_kernel_spmd(nc, in_maps, list(range(N_CORES)))
    return _assemble(res.results)
